# revision 21
# baseline (speedup 1.0000x reference)
"""Trainium2 Bass kernel for nn_Connection_75411035783724 (Mamba2 block + MLP head).

Single fused launch, tensor-parallel over the 32 Mamba2 heads across 8 cores
(4 heads each).  Per core: in_proj column-slice (x-channels + B + dt), causal
depthwise conv as accumulating diagonal matmuls, chunked-SSD scan (chunk 256).
The gated RMSNorm + out_proj + MLP tail runs in the same NEFF: each core
computes its out_proj partial on un-normalized gated outputs (the rsqrt
factors out of the contraction), one AllReduce sums partials + sum-of-squares,
then the normalizer is applied and the MLP (MLP2 column-sharded) finishes.

Schedule notes:
- conv matmuls for group g-1 are emitted after in_proj for group g so the PE
  never waits on same-group PSUM evictions; scan matmuls lag one batch.
- inter-chunk recurrence is matrix-form: per-chunk P[hp,c,pos] = S_c . C_pos
  matmuls run inside the loop; the decay-weight matrix W[h,c,pos] =
  exp(sum of chunk log-decays strictly between c and pos) is built post-loop
  with one fp32 matmul against a host-constant band indicator, broadcast to
  hp partitions by one-hot matmuls, then a multiply + log-tree c-reduction
  on the vector engine yields py directly (no big SBUF scan, no per-position
  matmuls, no DRAM round-trip).
- big loop buffers (conv output, transposed scan operands, chunk states) are
  2-batch rings; the freed SBUF holds 24/32 MLP1 weight tiles loaded during
  the loop, the rest + MLP2 stream under the AllReduce.
- partition-broadcasts ([4]->[128] etc.) are one-hot matmuls.
- every DRAM operand is host-pre-tiled to its exact SBUF layout so DMA loads
  are contiguous per partition.
"""
import os
import sys
import numpy as np
import ml_dtypes

sys.path.insert(0, "/opt/trn_rl_repo")

import concourse.bass as bass
import concourse.tile as tile
import concourse.mybir as mybir
from concourse import bacc
from concourse import bass_utils

F32 = mybir.dt.float32
BF16 = mybir.dt.bfloat16
AF = mybir.ActivationFunctionType
OP = mybir.AluOpType
BF = ml_dtypes.bfloat16

# Model dims
D_MODEL = 1024
HIDDEN = 4096
D_STATE = 128       # n
D_CONV = 4
D_INNER = 2048
HEADDIM = 64        # p
NHEADS = 32
CONV_DIM = D_INNER + 2 * D_STATE            # 2304
D_IN_PROJ = 2 * D_INNER + 2 * D_STATE + NHEADS  # 4384
L = 8192            # tokens
NPOS = 32           # output positions (first token of each frame)
POS_STRIDE = 256
NCORES = 8
HPC = 4             # heads per core
Q = 256             # chunk length
NCHUNK = L // Q     # 32
KT = D_MODEL // 128  # 8 K-tiles
NG = 16             # token groups of 512
GSZ = 512
BSZ = 2 * GSZ       # 1024-token batches for the decay pipe
NB = L // BSZ       # 8
CPB = BSZ // Q      # 4 chunks per batch
NCOL = 256 + 128  # 384: [x 256 | B 128]
MT_SPEC = [(0, 128), (128, 128), (256, 128)]  # (col0, width)
W1RES = 20          # MLP1 tiles resident in SBUF (loaded during the loop)


def _bf(x):
    return np.ascontiguousarray(np.asarray(x, dtype=np.float32)).astype(BF)


def _f32(x):
    return np.ascontiguousarray(np.asarray(x, dtype=np.float32))


_NC = None


def build():
    global _NC
    if _NC is not None:
        return _NC
    nc = bacc.Bacc("TRN2", target_bir_lowering=False, debug=False,
                   num_devices=NCORES)

    def din(name, shape, dt):
        return nc.dram_tensor(name, shape, dt, kind="ExternalInput").ap()

    xT = din("xT", (NG, 128, KT, GSZ), BF16)
    xTpos = din("xTpos", (128, KT, NPOS), BF16)
    xTwin = din("xTwin", (128, KT, NPOS * D_CONV), BF16)
    w_in = din("w_in", (128, KT, NCOL), BF16)
    w_dt = din("w_dt", (128, KT, HPC), BF16)
    w_c = din("w_c", (128, KT, 128), BF16)
    w_z = din("w_z", (128, KT, 256), BF16)
    diag_w = din("diag_w", (128, 3, D_CONV, 128), BF16)
    cw_c = din("cw_c", (128, D_CONV), F32)
    conv_b = din("conv_b", (128, 3), F32)
    conv_b_c = din("conv_b_c", (128, 1), F32)
    dtb4 = din("dtb4", (HPC, 1), F32)
    A4 = din("A4", (HPC, 1), F32)
    D4 = din("D4", (HPC, 1), F32)
    oh_w = din("oh_w", (HPC, 256), BF16)        # one-hot head->partition map
    wI = din("wI", (NCHUNK, NCHUNK * NPOS), F32)  # band indicator (c<k<=pos-1)
    maskneg = din("maskneg", (HPC, NCHUNK * NPOS), F32)  # -1e6 where c>=pos
    # tail
    nwl = din("nwl", (128, 2), F32)             # local norm weights
    wol = din("wol", (128, 2, D_MODEL), BF16)   # local out_proj rows
    w1 = din("w1T", (32, 128, KT, 128), BF16)
    b1 = din("b1", (128, HIDDEN // 128), F32)
    w2 = din("w2T", (4, 128, 32, 128), BF16)
    b2 = din("b2", (128, 4), F32)
    out32 = nc.dram_tensor("out32", (128, 4, NPOS), F32,
                           kind="ExternalOutput").ap()

    with tile.TileContext(nc) as tc:
        import contextlib
        with contextlib.ExitStack() as ctx:
            sb = ctx.enter_context(tc.tile_pool(name="sb", bufs=1))
            ring = ctx.enter_context(tc.tile_pool(name="ring", bufs=1))
            dsc = ctx.enter_context(tc.tile_pool(name="dsc", bufs=1, space="DRAM"))
            psA = ctx.enter_context(tc.tile_pool(name="psA", bufs=1, space="PSUM"))

            # ---- resident weights/constants (all pre-tiled, contiguous loads)
            w_in_sb = sb.tile([128, KT, NCOL], BF16)
            nc.sync.dma_start(out=w_in_sb, in_=w_in)
            w_dt_sb = sb.tile([128, KT, HPC], BF16)
            nc.sync.dma_start(out=w_dt_sb, in_=w_dt)
            xt_tiles = {}

            def load_xt(g):
                t = ring.tile([128, KT, GSZ], BF16, tag="xt", bufs=2)
                nc.sync.dma_start(out=t, in_=xT[g])
                xt_tiles[g] = t

            load_xt(0)
            load_xt(1)
            w_c_sb = sb.tile([128, KT, 128], BF16)
            nc.sync.dma_start(out=w_c_sb, in_=w_c)
            w_z_sb = sb.tile([128, KT, 256], BF16)
            nc.sync.dma_start(out=w_z_sb, in_=w_z)
            diag_sb = sb.tile([128, 3, D_CONV, 128], BF16)
            nc.sync.dma_start(out=diag_sb, in_=diag_w)
            cw_sb = sb.tile([128, D_CONV], F32)
            nc.sync.dma_start(out=cw_sb, in_=cw_c)
            cb_sb = sb.tile([128, 3], F32)
            nc.sync.dma_start(out=cb_sb, in_=conv_b)
            cbc_sb = sb.tile([128, 1], F32)
            nc.sync.dma_start(out=cbc_sb, in_=conv_b_c)
            dtb_sb = sb.tile([HPC, 1], F32)
            nc.sync.dma_start(out=dtb_sb, in_=dtb4)
            A_sb = sb.tile([HPC, 1], F32)
            nc.sync.dma_start(out=A_sb, in_=A4)
            D_sb = sb.tile([HPC, 1], F32)
            nc.sync.dma_start(out=D_sb, in_=D4)
            oh_sb = sb.tile([HPC, 256], BF16)
            nc.sync.dma_start(out=oh_sb, in_=oh_w)
            xtp_sb = sb.tile([128, KT, NPOS], BF16)
            nc.sync.dma_start(out=xtp_sb, in_=xTpos)
            xtw_sb = sb.tile([128, KT, NPOS * D_CONV], BF16)
            nc.sync.dma_start(out=xtw_sb, in_=xTwin)
            nwl_sb = sb.tile([128, 2], F32)
            nc.sync.dma_start(out=nwl_sb, in_=nwl)
            wol_sb = sb.tile([128, 2, D_MODEL], BF16)
            nc.scalar.dma_start(out=wol_sb, in_=wol)
            b1_sb = sb.tile([128, HIDDEN // 128], F32)
            nc.sync.dma_start(out=b1_sb, in_=b1)
            b2_sb = sb.tile([128, 4], F32)
            nc.sync.dma_start(out=b2_sb, in_=b2)
            wI_sb = sb.tile([NCHUNK, NCHUNK * NPOS], F32)
            nc.scalar.dma_start(out=wI_sb, in_=wI)
            mneg_sb = sb.tile([HPC, NCHUNK * NPOS], F32)
            nc.scalar.dma_start(out=mneg_sb, in_=maskneg)

            # ---- MLP weights: 24/32 MLP1 tiles resident, loaded mid-loop
            w1_sb = sb.tile([128, W1RES, KT, 128], BF16)

            # ---- persistent buffers
            P_sb = sb.tile([128, 2, NCHUNK * NPOS], BF16)  # S_c . C_pos
            dt_pos = sb.tile([HPC, NCHUNK], F32)     # dt at chunk starts
            a_pos = sb.tile([HPC, NCHUNK], F32)      # a = dt*A at chunk starts
            x32 = sb.tile([128, 2, NPOS], F32)
            B32 = sb.tile([128, NPOS], F32)
            ones4 = sb.tile([HPC, BSZ], F32)
            one4 = sb.tile([HPC, 1], F32)
            dt2_slots = [sb.tile([HPC, BSZ], F32, tag="dt2a", name="dt2a"),
                         sb.tile([HPC, BSZ], F32, tag="dt2b", name="dt2b")]
            a2_s = sb.tile([HPC, BSZ], F32)
            s2_s = sb.tile([HPC, BSZ], F32)
            w2b_s = sb.tile([HPC, BSZ], BF16)
            qsT_d = dsc.tile([NCHUNK, HPC], F32)     # chunk log-decays, c-major
            onesc = sb.tile([128, 1], BF16)
            oh14 = sb.tile([1, HPC], BF16)
            ones128b = sb.tile([1, 128], BF16)
            hp_all = sb.tile([128, 9, NPOS], BF16)
            nc.vector.memset(onesc, 1.0)
            nc.vector.memset(oh14, 1.0)
            nc.vector.memset(ones128b, 1.0)
            nc.vector.memset(hp_all[:, 8, :], 0.0)
            nc.vector.memset(ones4, 1.0)
            nc.vector.memset(one4, 1.0)
            z_ap = bass.AP(tensor=ones4.tensor, offset=ones4.offset,
                           ap=[list(ones4.ap[0]), [Q, CPB]])
            nc.vector.memset(z_ap, 0.0)

            # ================= pre-loop tail-independent work ================
            C32 = sb.tile([128, NPOS], F32)
            pcw = psA.tile([128, NPOS * D_CONV], F32, tag="pin0")
            for k in range(KT):
                nc.tensor.matmul(pcw, w_c_sb[:, k, :], xtw_sb[:, k, :],
                                 start=(k == 0), stop=(k == KT - 1))
            tmpc = sb.tile([128, NPOS], F32)
            for j in range(D_CONV):
                src = bass.AP(tensor=pcw.tensor, offset=pcw.offset + j,
                              ap=[list(pcw.ap[0]), [D_CONV, NPOS]])
                if j == 0:
                    nc.vector.tensor_scalar_mul(tmpc, src, cw_sb[:, 0:1])
                else:
                    nc.vector.scalar_tensor_tensor(
                        out=tmpc, in0=src, scalar=cw_sb[:, j:j + 1], in1=tmpc,
                        op0=OP.mult, op1=OP.add)
            nc.scalar.activation(out=C32, in_=tmpc, func=AF.Silu,
                                 bias=cbc_sb[:, 0:1], scale=1.0)
            C32b = sb.tile([128, NPOS], BF16)
            nc.vector.tensor_copy(out=C32b, in_=C32)
            pz = psA.tile([128, 2, NPOS], F32, tag="pin1")
            for t in range(2):
                for k in range(KT):
                    nc.tensor.matmul(pz[:, t, :],
                                     w_z_sb[:, k, t * 128:(t + 1) * 128],
                                     xtp_sb[:, k, :],
                                     start=(k == 0), stop=(k == KT - 1))
            zs = sb.tile([128, 2, NPOS], F32)
            nc.scalar.activation(out=zs, in_=pz, func=AF.Silu)

            # ================= main fused loop =================
            pins = {}
            pdts = {}
            xbc_tiles = {}
            xbcc_tiles = {}   # conv+silu output, per-batch ring
            XT_tiles = {}
            BT_tiles = {}

            def emit_inproj(g):
                xt_g = xt_tiles.pop(g)
                ps = []
                for mt, (c0, cw) in enumerate(MT_SPEC):
                    p = psA.tile([cw, GSZ], F32, tag=f"pin{mt}")
                    for k in range(KT):
                        nc.tensor.matmul(p, w_in_sb[:, k, c0:c0 + cw],
                                         xt_g[:, k, :],
                                         start=(k == 0), stop=(k == KT - 1))
                    ps.append(p)
                pdt = psA.tile([HPC, GSZ], F32, tag="pdt", bufs=2)
                for k in range(KT):
                    nc.tensor.matmul(pdt, w_dt_sb[:, k, :], xt_g[:, k, :],
                                     start=(k == 0), stop=(k == KT - 1))
                pins[g] = ps
                pdts[g] = pdt

            def emit_evict(g):
                ps = pins.pop(g)
                xbc_g = ring.tile([128, 3, GSZ + 3], BF16, tag="xbc", bufs=2)
                if g == 0:
                    nc.vector.memset(xbc_g[:, :, 0:3], 0.0)
                else:
                    prev = xbc_tiles[g - 1]
                    nc.vector.tensor_copy(out=xbc_g[:, :, 0:3],
                                          in_=prev[:, :, GSZ:GSZ + 3])
                for cht in range(3):
                    if cht != 2:
                        nc.vector.tensor_copy(out=xbc_g[:, cht, 3:], in_=ps[cht])
                    else:
                        nc.scalar.copy(out=xbc_g[:, cht, 3:], in_=ps[cht])
                xbc_tiles[g] = xbc_g
                b, half = divmod(g, 2)
                dt2 = dt2_slots[b % 2]
                pdt = pdts.pop(g)
                nc.vector.tensor_copy(out=dt2[:, half * GSZ:(half + 1) * GSZ],
                                      in_=pdt)

            def emit_conv(g):
                xbc_g = xbc_tiles[g]
                b, half = divmod(g, 2)
                if half == 0:
                    xbcc_tiles[b] = ring.tile([128, 3, BSZ], BF16,
                                              tag="xbcc", bufs=2, name="xbcc")
                xbcc_b = xbcc_tiles[b]
                sl = slice(half * GSZ, (half + 1) * GSZ)
                for cht in range(3):
                    pc = psA.tile([128, GSZ], F32, tag="psh", bufs=3)
                    for j in range(D_CONV):
                        nc.tensor.matmul(pc, diag_sb[:, cht, j, :],
                                         xbc_g[:, cht, j:j + GSZ],
                                         start=(j == 0), stop=(j == D_CONV - 1))
                    nc.scalar.activation(out=xbcc_b[:, cht, sl], in_=pc,
                                         func=AF.Silu,
                                         bias=cb_sb[:, cht:cht + 1], scale=1.0)

            def emit_decay(b):
                dt2 = dt2_slots[b % 2]
                # softplus: dt = ln(1 + exp(v + bias))
                nc.scalar.activation(out=a2_s, in_=dt2, func=AF.Exp,
                                     bias=dtb_sb[:, 0:1], scale=1.0)
                nc.scalar.activation(out=dt2, in_=a2_s, func=AF.Ln,
                                     bias=one4[:, 0:1], scale=1.0)
                a2 = a2_s
                nc.vector.tensor_scalar_mul(a2, dt2, A_sb[:, 0:1])
                s2 = s2_s
                nc.vector.tensor_tensor_scan(out=s2, data0=ones4, data1=a2,
                                             initial=0.0, op0=OP.mult, op1=OP.add)
                cpos = b * CPB
                # a at chunk starts (exp'd post-loop for dA)
                src = bass.AP(tensor=a2.tensor, offset=a2.offset,
                              ap=[list(a2.ap[0]), [Q, CPB]])
                nc.vector.tensor_copy(out=a_pos[:, cpos:cpos + CPB], in_=src)
                src = bass.AP(tensor=dt2.tensor, offset=dt2.offset,
                              ap=[list(dt2.ap[0]), [Q, CPB]])
                nc.vector.tensor_copy(out=dt_pos[:, cpos:cpos + CPB], in_=src)
                # chunk log-decays q_c -> DRAM, transposed (c-major) for the
                # post-loop W build
                src = bass.AP(tensor=s2.tensor, offset=s2.offset + Q - 1,
                              ap=[list(s2.ap[0]), [Q, CPB]])
                dst = bass.AP(tensor=qsT_d.tensor,
                              offset=qsT_d.offset + cpos * HPC,
                              ap=[[1, HPC], [HPC, CPB]])
                nc.gpsimd.dma_start(out=dst, in_=src)
                # w = exp(stot - s) * dt
                for cc in range(CPB):
                    stot = bass.AP(tensor=s2.tensor,
                                   offset=s2.offset + cc * Q + Q - 1,
                                   ap=[list(s2.ap[0]), [1, 1]])
                    nc.vector.tensor_scalar(s2[:, cc * Q:(cc + 1) * Q],
                                            s2[:, cc * Q:(cc + 1) * Q],
                                            stot, None, OP.subtract)
                nc.scalar.activation(out=s2, in_=s2, func=AF.Exp, scale=-1.0)
                nc.vector.tensor_mul(w2b_s, s2, dt2)
                return w2b_s

            def emit_scale(b, w2b):
                xbcc_b = xbcc_tiles[b]
                XT_b = ring.tile([128, 8, 256], BF16, tag="XT", bufs=2)
                BT_b = ring.tile([128, 8, 128], BF16, tag="BT", bufs=2)
                XT_tiles[b] = XT_b
                BT_tiles[b] = BT_b
                for half in range(2):
                    hsl = slice(half * GSZ, (half + 1) * GSZ)
                    for t in range(2):
                        pw = psA.tile([128, GSZ], F32, tag="psh", bufs=3)
                        nc.tensor.matmul(pw, oh_sb[:, t * 128:(t + 1) * 128],
                                         w2b[:, hsl], start=True, stop=True)
                        xs = ring.tile([128, GSZ], BF16, tag="xs", bufs=2)
                        nc.vector.tensor_mul(xs, xbcc_b[:, t, hsl], pw)
                        nc.sync.dma_start_transpose(
                            out=XT_b[:, 4 * half:4 * (half + 1),
                                     t * 128:(t + 1) * 128],
                            in_=xs)
                nc.sync.dma_start_transpose(out=BT_b, in_=xbcc_b[:, 2, :])
                # position extracts for this batch (4 positions per batch)
                for cht in range(2):
                    s_ap = bass.AP(tensor=xbcc_b.tensor,
                                   offset=xbcc_b.offset + cht * BSZ,
                                   ap=[list(xbcc_b.ap[0]), [POS_STRIDE, CPB]])
                    nc.gpsimd.tensor_copy(out=x32[:, cht, CPB * b:CPB * (b + 1)],
                                          in_=s_ap)
                s_ap = bass.AP(tensor=xbcc_b.tensor,
                               offset=xbcc_b.offset + 2 * BSZ,
                               ap=[list(xbcc_b.ap[0]), [POS_STRIDE, CPB]])
                nc.gpsimd.tensor_copy(out=B32[:, CPB * b:CPB * (b + 1)],
                                      in_=s_ap)

            def emit_scan(b):
                XT_b = XT_tiles.pop(b)
                BT_b = BT_tiles.pop(b)
                xbcc_tiles.pop(b)
                S_r = ring.tile([128, 256 * CPB], BF16, tag="sring", bufs=2)
                for cc in range(CPB):
                    pc2 = psA.tile([128, HPC * HEADDIM], F32, tag="psh", bufs=3)
                    for k2 in range(2):
                        T = 2 * cc + k2
                        nc.tensor.matmul(pc2, BT_b[:, T, :], XT_b[:, T, :],
                                         start=(k2 == 0), stop=(k2 == 1))
                    dst = bass.AP(tensor=S_r.tensor, offset=S_r.offset + cc,
                                  ap=[list(S_r.ap[0]), [CPB, HPC * HEADDIM]])
                    if cc % 2 == 0:
                        nc.vector.tensor_copy(out=dst, in_=pc2)
                    else:
                        nc.scalar.copy(out=dst, in_=pc2)
                # P[hp, c, pos] = S_c . C_pos for this batch's 4 chunks
                # (both t-halves share one PSUM bank to ease pdt pressure)
                P_ps = psA.tile([128, 2, CPB, NPOS], F32, tag="pdt", bufs=2)
                for t in range(2):
                    for cc in range(CPB):
                        lhs = bass.AP(tensor=S_r.tensor,
                                      offset=S_r.offset + t * 512 + cc,
                                      ap=[list(S_r.ap[0]), [CPB, 128]])
                        nc.tensor.matmul(P_ps[:, t, cc, :], lhs, C32b,
                                         start=True, stop=True,
                                         skip_group_check=True)
                for t in range(2):
                    dstP = P_sb[:, t, b * CPB * NPOS:(b + 1) * CPB * NPOS]
                    if t == 0:
                        nc.vector.tensor_copy(out=dstP, in_=P_ps[:, t])
                    else:
                        nc.scalar.copy(out=dstP, in_=P_ps[:, t])

            for g in range(NG + 2):
                if g + 2 < NG:
                    load_xt(g + 2)
                if 5 <= g < 5 + W1RES // 2:
                    mt0 = (g - 5) * 2
                    nc.scalar.dma_start(
                        out=w1_sb[:, mt0:mt0 + 2],
                        in_=w1[mt0:mt0 + 2].rearrange("m p k c -> p m k c"))
                if g < NG:
                    emit_inproj(g)
                    emit_evict(g)
                if g >= 1 and g - 1 < NG:
                    emit_conv(g - 1)
                if g >= 2 and g % 2 == 0:
                    b = (g - 2) // 2
                    emit_scale(b, emit_decay(b))
                if g >= 3 and g % 2 == 1:
                    emit_scan((g - 3) // 2)

            # ================= tail =================
            # stream the remaining MLP weights under the W build + AllReduce
            w1_tiles = [None] * 32

            def load_w1(mt):
                t = ring.tile([128, KT, 128], BF16, tag="w1r", bufs=32 - W1RES)
                nc.sync.dma_start(out=t, in_=w1[mt])
                w1_tiles[mt] = t

            w2_tiles = [None] * 4

            def load_w2(mt, eng=None):
                t = ring.tile([128, 32, 128], BF16, tag="w2r", bufs=2)
                (eng or nc.sync).dma_start(out=t, in_=w2[mt])
                w2_tiles[mt] = t

            # all fresh ring slots: no WAR waits, loads drain under the
            # collective
            for mt in range(W1RES, 32):
                load_w1(mt)
            load_w2(0)
            load_w2(1)

            # ---- decay-weight matrix W[h, (c,pos)] = exp(I^T q + mask)
            qT_sb = sb.tile([NCHUNK, HPC], F32)
            nc.gpsimd.dma_start(out=qT_sb, in_=qsT_d)
            E_sb = sb.tile([HPC, NCHUNK * NPOS], F32)
            for half in range(2):
                hs = slice(half * 512, (half + 1) * 512)
                pE = psA.tile([HPC, 512], F32, tag="psh", bufs=3)
                nc.tensor.matmul(pE, qT_sb, wI_sb[:, hs], start=True, stop=True)
                nc.vector.tensor_add(E_sb[:, hs], pE, mneg_sb[:, hs])
            W4b = sb.tile([HPC, NCHUNK * NPOS], BF16)
            nc.scalar.activation(out=W4b, in_=E_sb, func=AF.Exp)
            # broadcast W to hp partitions and reduce over c
            py_sb = sb.tile([128, 2, NPOS], F32)
            pwt = [sb.tile([128, 512], F32, name=f"pwt{i}") for i in range(2)]
            for t in range(2):
                psW = []
                for half in range(2):
                    ptag = ("pin0", "pin1", "pin2", "pdt")[2 * t + half]
                    pW = psA.tile([128, 512], F32, tag=ptag,
                                  bufs=2 if ptag == "pdt" else 1)
                    nc.tensor.matmul(pW, oh_sb[:, t * 128:(t + 1) * 128],
                                     W4b[:, half * 512:(half + 1) * 512],
                                     start=True, stop=True)
                    psW.append(pW)
                for half in range(2):
                    nc.vector.tensor_mul(
                        pwt[half],
                        P_sb[:, t, half * 512:(half + 1) * 512], psW[half])
                nc.vector.tensor_add(pwt[0], pwt[0], pwt[1])      # fold c 16
                nc.vector.tensor_add(pwt[0][:, 0:256], pwt[0][:, 0:256],
                                     pwt[0][:, 256:512])          # fold c 8
                nc.vector.tensor_add(pwt[0][:, 0:128], pwt[0][:, 0:128],
                                     pwt[0][:, 128:256])          # fold c 4
                nc.vector.tensor_add(pwt[0][:, 0:64], pwt[0][:, 0:64],
                                     pwt[0][:, 64:128])           # fold c 2
                nc.vector.tensor_add(py_sb[:, t, :], pwt[0][:, 0:32],
                                     pwt[0][:, 32:64])            # fold c 1

            # ---- f4 = dt_pos*(B32.C32)+D; dA = exp(a_pos); fold the z-gate
            dAzs = sb.tile([128, 2, NPOS], F32)
            tloc2 = sb.tile([128, 2, NPOS], F32)
            bc_t = sb.tile([128, NPOS], BF16)
            nc.vector.tensor_mul(bc_t, B32, C32)
            pbc = psA.tile([1, NPOS], F32, tag="pdt", bufs=2)
            nc.tensor.matmul(pbc, onesc, bc_t, start=True, stop=True)
            bc_row = sb.tile([1, NPOS], BF16)
            nc.scalar.copy(out=bc_row, in_=pbc)
            pbc4 = psA.tile([HPC, NPOS], F32, tag="pdt", bufs=2)
            nc.tensor.matmul(pbc4, oh14, bc_row, start=True, stop=True)
            f4 = sb.tile([HPC, NPOS], F32)
            nc.vector.tensor_mul(f4, dt_pos, pbc4)
            nc.vector.tensor_scalar(f4, f4, D_sb[:, 0:1], None, OP.add)
            f4b = sb.tile([HPC, NPOS], BF16)
            nc.scalar.copy(out=f4b, in_=f4)
            dAAb = sb.tile([HPC, NCHUNK], BF16)
            nc.scalar.activation(out=dAAb, in_=a_pos, func=AF.Exp)
            pda = psA.tile([128, 2, NPOS], F32, tag="pin0")
            pf = psA.tile([128, 2, NPOS], F32, tag="pin1")
            for t in range(2):
                nc.tensor.matmul(pda[:, t, :],
                                 oh_sb[:, t * 128:(t + 1) * 128],
                                 dAAb, start=True, stop=True)
                nc.tensor.matmul(pf[:, t, :],
                                 oh_sb[:, t * 128:(t + 1) * 128],
                                 f4b, start=True, stop=True)
            nc.vector.tensor_mul(dAzs, pda, zs)
            nc.vector.tensor_mul(tloc2, x32, pf)
            nc.vector.tensor_mul(tloc2, tloc2, zs)

            # y = py*(dA*zs) + (x32*f*zs), then y*norm_w (un-normalized)
            y32 = sb.tile([128, 2, NPOS], F32)
            nc.vector.tensor_mul(y32, py_sb, dAzs)
            nc.vector.tensor_add(y32, y32, tloc2)
            sq2 = sb.tile([128, 2, NPOS], BF16)
            nc.vector.tensor_mul(sq2, y32, y32)
            ynwb = sb.tile([128, 2, NPOS], BF16)
            for t in range(2):
                nc.vector.tensor_scalar_mul(ynwb[:, t, :], y32[:, t, :],
                                            nwl_sb[:, t:t + 1])

            # local out_proj partials + sum-of-squares -> AllReduce buffer
            arb = dsc.tile([128, 9, NPOS], BF16)
            for mt in range(8):
                php = psA.tile([128, NPOS], F32, tag="psh", bufs=3)
                for t in range(2):
                    nc.tensor.matmul(php, wol_sb[:, t, mt * 128:(mt + 1) * 128],
                                     ynwb[:, t, :], start=(t == 0), stop=(t == 1))
                if mt % 2 == 0:
                    nc.vector.tensor_copy(out=hp_all[:, mt, :], in_=php)
                else:
                    nc.scalar.copy(out=hp_all[:, mt, :], in_=php)
            pss = psA.tile([1, NPOS], F32, tag="pdt", bufs=2)
            for t in range(2):
                nc.tensor.matmul(pss, onesc, sq2[:, t, :],
                                 start=(t == 0), stop=(t == 1))
            nc.vector.tensor_copy(out=hp_all[0:1, 8, :], in_=pss)
            nc.gpsimd.dma_start(out=arb, in_=hp_all)

            # warm the ln/exp activation table under the collective so the
            # post-AllReduce rsqrt chain doesn't pay the table load
            eps_t = sb.tile([1, 1], F32)
            nc.vector.memset(eps_t, 1e-5)
            dumE = sb.tile([1, 1], F32)
            nc.scalar.activation(out=dumE, in_=eps_t, func=AF.Exp)

            arb_out = dsc.tile([128, 9, NPOS], BF16)
            nc.gpsimd.collective_compute(
                "AllReduce", mybir.AluOpType.add,
                replica_groups=[list(range(NCORES))],
                ins=[arb.opt()], outs=[arb_out.opt()],
            )
            hsum = sb.tile([128, 9, NPOS], BF16)
            nc.sync.dma_start(out=hsum, in_=arb_out)
            # w2[2]/w2[3] ride the otherwise-idle gpsimd queue; their ring
            # slots free after the first two MLP2 steps read w2[0]/w2[1]
            load_w2(2, eng=nc.gpsimd)
            load_w2(3, eng=nc.gpsimd)

            # r = 1/sqrt(mean + eps) = exp(-0.5*ln(mean + eps)).  The MLP1
            # matmuls run on the un-normalized hsum (r commutes with the
            # linear layer, applied per-position before the gelu) so they
            # start the moment hsum lands.
            rs = sb.tile([1, NPOS], F32)
            nc.scalar.activation(out=rs, in_=hsum[0:1, 8, :], func=AF.Ln,
                                 bias=eps_t[:, 0:1], scale=1.0 / D_INNER)
            rsb = sb.tile([1, NPOS], BF16)
            nc.scalar.activation(out=rsb, in_=rs, func=AF.Exp, scale=-0.5)
            prs = psA.tile([128, NPOS], F32, tag="pdt", bufs=2)
            nc.tensor.matmul(prs, ones128b, rsb, start=True, stop=True)
            r_bc = sb.tile([128, NPOS], F32)
            nc.vector.tensor_copy(out=r_bc, in_=prs)

            # ---- g = gelu(r * (w1T.T @ hsum) + b1)  [4096, 32]
            g_sb = sb.tile([128, 32, NPOS], BF16)
            for mt in range(32):
                wsrc = w1_sb[:, mt, :, :] if mt < W1RES else w1_tiles[mt]
                pg = psA.tile([128, NPOS], F32, tag="psh", bufs=3)
                for k in range(KT):
                    nc.tensor.matmul(pg, wsrc[:, k, :],
                                     hsum[:, k, :], start=(k == 0),
                                     stop=(k == KT - 1))
                gt = ring.tile([128, NPOS], F32, tag="gt", bufs=2)
                nc.vector.tensor_mul(gt, pg, r_bc)
                nc.scalar.activation(out=g_sb[:, mt, :], in_=gt, func=AF.Gelu,
                                     bias=b1_sb[:, mt:mt + 1], scale=1.0)
            # ---- out = w2T.T @ g + b2   [512, 32] per core
            for mt in range(4):
                po = psA.tile([128, NPOS], F32, tag="psh", bufs=3)
                for k in range(32):
                    nc.tensor.matmul(po, w2_tiles[mt][:, k, :],
                                     g_sb[:, k, :], start=(k == 0), stop=(k == 31))
                ot = sb.tile([128, NPOS], F32, tag="ot", bufs=2)
                nc.vector.tensor_scalar(ot, po, b2_sb[:, mt:mt + 1], None, OP.add)
                nc.sync.dma_start(out=out32[:, mt, :], in_=ot)

    nc.compile()
    _NC = nc
    return nc


# ----------------------------------------------------------------------------
# Host-side prep + glue
# ----------------------------------------------------------------------------

def _prep_maps(inputs):
    x = _f32(inputs["x"]).reshape(L, D_MODEL)
    xT = np.ascontiguousarray(x.T)                       # [1024, 8192]
    # pre-tiled xT: [NG, 128, KT, GSZ]
    xT_t = _bf(xT.reshape(KT, 128, NG, GSZ).transpose(2, 1, 0, 3))
    pos = np.arange(NPOS) * POS_STRIDE
    xTpos = _bf(xT[:, pos].reshape(KT, 128, NPOS).transpose(1, 0, 2))
    win_idx = (pos[:, None] + np.arange(D_CONV)[None, :] - (D_CONV - 1)).reshape(-1)
    xTwin = np.zeros((D_MODEL, NPOS * D_CONV), np.float32)
    valid = win_idx >= 0
    xTwin[:, valid] = xT[:, win_idx[valid]]
    xTwin = _bf(xTwin.reshape(KT, 128, NPOS * D_CONV).transpose(1, 0, 2))

    w_all = _f32(inputs["in_proj_w"])                    # [4384, 1024]
    conv_w = _f32(inputs["conv_w"])                      # [2304, 4]
    conv_b = _f32(inputs["conv_b"])                      # [2304]
    dt_bias = _f32(inputs["dt_bias"])                    # [32]
    A = -np.exp(_f32(inputs["A_log"]))                   # [32]
    Dp = _f32(inputs["D"])                               # [32]

    w_cT = _bf(w_all[D_INNER + D_INNER + D_STATE:
                     D_INNER + D_INNER + 2 * D_STATE]
               .T.reshape(KT, 128, 128).transpose(1, 0, 2))
    cw_c = _f32(conv_w[D_INNER + D_STATE:])              # [128, 4] C channels
    conv_b_c = _f32(conv_b[D_INNER + D_STATE:]).reshape(128, 1)

    oh = np.zeros((HPC, 256), np.float32)
    for t in range(2):
        for p in range(128):
            oh[2 * t + p // 64, t * 128 + p] = 1.0
    oh = _bf(oh)

    # band indicator I[k, (c,pos)] = 1 if c < k <= pos-1; mask -1e6 at c >= pos
    kk = np.arange(NCHUNK)[:, None, None]
    cc_i = np.arange(NCHUNK)[None, :, None]
    pp = np.arange(NPOS)[None, None, :]
    wI = ((cc_i < kk) & (kk <= pp - 1)).astype(np.float32)
    wI = wI.reshape(NCHUNK, NCHUNK * NPOS).copy()
    mn = np.where(cc_i[0] >= pp[0], np.float32(-1e6), np.float32(0.0))
    maskneg = np.broadcast_to(mn.reshape(1, -1), (HPC, NCHUNK * NPOS)).copy()

    norm_w = _f32(inputs["norm_w"])                      # [2048]
    w_out = _f32(inputs["mamba_out_w"])                  # [1024, 2048]
    w1 = _f32(inputs["mlp_w1"])                          # [4096, 1024]
    w1_t = _bf(w1.T.reshape(KT, 128, 32, 128).transpose(2, 1, 0, 3))
    b1 = _f32(inputs["mlp_b1"]).reshape(32, 128).transpose(1, 0).copy()
    w2 = _f32(inputs["mlp_w2"])                          # [4096, 4096]

    maps = []
    for k in range(NCORES):
        xs = 256 * k
        cols = np.concatenate([
            np.arange(D_INNER + xs, D_INNER + xs + 256),          # x slice
            np.arange(2 * D_INNER, 2 * D_INNER + D_STATE),        # B
        ])
        w_in = _bf(w_all[cols].T.reshape(KT, 128, NCOL).transpose(1, 0, 2))
        dt_cols = np.arange(D_IN_PROJ - NHEADS + HPC * k,
                            D_IN_PROJ - NHEADS + HPC * k + HPC)
        w_dtp = _bf(w_all[dt_cols].T.reshape(KT, 128, HPC).transpose(1, 0, 2))
        w_z = _bf(w_all[xs:xs + 256].T.reshape(KT, 128, 256).transpose(1, 0, 2))
        ch_x = np.arange(xs, xs + 256)
        ch_B = np.arange(D_INNER, D_INNER + D_STATE)
        dw = np.zeros((3, D_CONV, 128, 128), np.float32)
        cb = np.zeros((128, 3), np.float32)
        for cht, chs in enumerate([ch_x[:128], ch_x[128:], ch_B]):
            for j in range(D_CONV):
                dw[cht, j] = np.diag(conv_w[chs, j])
            cb[:, cht] = conv_b[chs]
        dw = _bf(dw.transpose(2, 0, 1, 3))               # [128, 3, 4, 128]
        heads = np.arange(HPC * k, HPC * k + HPC)
        # local channels of this core: ch = 256k + t*128 + p
        chl = (xs + np.arange(256)).reshape(2, 128)      # [t, p]
        nwl = norm_w[chl].T.copy()                       # [128, 2]
        wol = _bf(w_out[:, xs:xs + 256].T.reshape(2, 128, D_MODEL)
                  .transpose(1, 0, 2))                   # [128, 2, 1024]
        colsl = slice(512 * k, 512 * k + 512)
        w2_t = _bf(w2[colsl].T.reshape(32, 128, 4, 128).transpose(2, 1, 0, 3))
        b2 = _f32(inputs["mlp_b2"])[colsl].reshape(4, 128).transpose(1, 0).copy()
        maps.append({
            "xT": xT_t, "xTpos": xTpos, "xTwin": xTwin,
            "w_in": w_in, "w_dt": w_dtp, "w_c": w_cT, "w_z": w_z,
            "diag_w": dw, "cw_c": cw_c, "conv_b": cb,
            "conv_b_c": conv_b_c,
            "dtb4": dt_bias[heads].reshape(HPC, 1).astype(np.float32),
            "A4": A[heads].reshape(HPC, 1).astype(np.float32),
            "D4": Dp[heads].reshape(HPC, 1).astype(np.float32),
            "oh_w": oh, "wI": wI, "maskneg": maskneg,
            "nwl": nwl, "wol": wol, "w1T": w1_t, "b1": b1,
            "w2T": w2_t, "b2": b2,
        })
    return maps


LAST_RESULTS = []


def kernel(**inputs) -> np.ndarray:
    trace = os.environ.get("KERNEL_TRACE", "0") == "1"
    LAST_RESULTS.clear()
    nc = build()
    maps = _prep_maps(inputs)
    kw = {}
    if os.environ.get("KERNEL_TRACE_ALL", "0") == "1":
        kw["trace_cores"] = list(range(NCORES))
    res = bass_utils.run_bass_kernel_spmd(nc, maps, core_ids=list(range(NCORES)),
                                          trace=trace, **kw)
    LAST_RESULTS.append(res)
    out = np.zeros((NPOS, HIDDEN), np.float32)
    for k in range(NCORES):
        o = res.results[k]["out32"]                     # [128, 4, 32]
        out[:, 512 * k:512 * (k + 1)] = o.transpose(2, 1, 0).reshape(NPOS, 512)
    return out.astype(np.float32)
